# revision 62
# baseline (speedup 1.0000x reference)
"""ANOVA kernel (order 3) on 8 TRN2 NeuronCores.

Math: out[b] = sum_e e3(x[b, :, e]) where e3 is the 3rd elementary
symmetric polynomial over the field axis. Via Newton's identities:
    e3 = (p1^3 - 3*p1*p2 + 2*p3) / 6,   p_k = sum_f x^k
so the kernel is: elementwise x^2 (ScalarE), x^3 (VectorE), a parity
pre-sum of x (VectorE), field-axis reductions on TensorE (matmuls with
one-hot selector weights, bf16), then a small fused finale.

The host pre-casts x to bf16 (the bf16 pipeline keeps rel-err ~3e-3,
well under tolerance) and pre-permutes each core's shard into the
on-chip layout, so every SBUF partition reads a contiguous DRAM run:
the DMA moves 4 KB descriptors at full HBM rate instead of 256 B
strided chunks.

Data parallel over batch: core c handles b in [1024*c, 1024*(c+1)).

Layout per core: tile tau covers 16 consecutive b. SBUF tile (128, 512)
bf16: partition p = b_q*32 + fp (b_q in [0,4), f-pair fp in [0,32)),
free n = j2*128 + parity*64 + e. DRAM x_perm[p, tau*512 + n] is this
exact stream, so superblock loads are plain 2D slices.

A matmul with a 32-column one-hot selector lhsT (col m one-hot at
m = 4*(tau' % 8) + b_q) accumulates each tile's f-pair sums into PSUM
rows 4*tau' + b_q of a 32-row block; 32 tiles fill a (128, 512) PSUM
tensor per stat (one fill per half of the core's batch). p1 uses a
parity-presummed rhs (s1 = x_par0 + x_par1, VectorE) at half free
size. The finale combines p2's two f-parity halves, applies Newton's
formula, and reduces over e. The first fill's finale runs mid-kernel,
hidden under DMA.
"""

import sys

if "/opt/trn_rl_repo" not in sys.path:
    sys.path.insert(0, "/opt/trn_rl_repo")

import numpy as np

N_CORES = 8
B, F, E = 8192, 64, 64
B_PER_CORE = B // N_CORES  # 1024
J2 = 4                     # b-quads per tile
FD = 512                   # tile free dim = J2 * 2 * E (one PSUM bank)
TILES = B_PER_CORE // 16   # 64 (16 b per tile)
SUPER = 4                  # tiles per superblock for big ACT/DVE ops
N_SUPER = TILES // SUPER   # 16
SFD = FD * SUPER           # 2048
FILL_SUPERS = 8            # supers per PSUM fill (32 tiles = 128 rows)
WARMUP_MMS = 32            # dummy matmuls to lift the PE HAM clock gate

_cache = {}


def _make_g() -> np.ndarray:
    """One-hot selector weights (128, 124) bf16: row k has a 1 at col
    60 + k//32. lhsT for tile tau' is g[:, 60-4*m16 : 124-4*m16] with
    m16 = tau' % 16, so lhsT[k, m] = 1 iff m == 4*m16 + k//32."""
    import ml_dtypes

    g = np.zeros((128, 124), dtype=ml_dtypes.bfloat16)
    for k in range(128):
        g[k, 60 + k // 32] = 1.0
    return g


def _build():
    import concourse.bass as bass
    import concourse.tile as tile
    from concourse import bacc, mybir

    nc = bacc.Bacc(
        "TRN2", target_bir_lowering=False, debug=False, num_devices=N_CORES
    )
    f32 = mybir.dt.float32
    bf16 = mybir.dt.bfloat16

    x_dram = nc.dram_tensor(
        "x", [128, TILES * FD], bf16, kind="ExternalInput"
    ).ap()
    g_dram = nc.dram_tensor("g", [128, 124], bf16, kind="ExternalInput").ap()
    out_dram = nc.dram_tensor("out", [128, 2 * J2], f32, kind="ExternalOutput").ap()

    with tile.TileContext(nc) as tc:
        with (
            tc.tile_pool(name="const", bufs=1) as const_pool,
            tc.tile_pool(name="xin", bufs=12) as x_pool,
            tc.tile_pool(name="xsq", bufs=8) as x2_pool,
            tc.tile_pool(name="xcu", bufs=8) as x3_pool,
            tc.tile_pool(name="xs1", bufs=8) as s1_pool,
            tc.tile_pool(name="acc", bufs=1, space="PSUM") as psum_pool,
            tc.tile_pool(name="tail", bufs=1) as tail_pool,
        ):
            g_sb = const_pool.tile([128, 124], bf16)
            outt = const_pool.tile([128, 2 * J2], f32)

            # p1 accumulates parity-presummed rhs (free dim FD//2); p2/p3
            # accumulate full-parity rhs (free dim FD). Folding p2's
            # parities on DVE too measures worse: the strided parity
            # pre-add runs at ~933ns (1x mode), costing DVE more than the
            # matmul columns it saves. Splitting a stat into 2x N=256
            # parity matmuls is also ~12% slower (long N=256 bursts
            # become LDWEIGHTS-bound).
            psums = [
                [
                    psum_pool.tile(
                        [128, FD // 2 if stat == 0 else FD],
                        f32,
                        name=f"psum_{phi}_{stat}",
                    )
                    for stat in range(3)
                ]
                for phi in range(2)
            ]

            # PE warmup: the HAM clock gate holds the PE at 1.2 GHz until
            # it has seen ~3.4us of sustained activity. Dummy matmuls on a
            # zeroed tile during the DMA ramp put the array at 2.4 GHz
            # before the first real matmul issues.
            zb = const_pool.tile([128, FD], bf16)
            warm_ps = psum_pool.tile([128, FD], f32, name="warm")
            nc.gpsimd.memset(zb[:], 0.0)
            for _ in range(WARMUP_MMS):
                nc.tensor.matmul(
                    warm_ps[0:64, :128], zb[:, :64], zb[:, :128],
                    start=True, stop=True, skip_group_check=True,
                )

            def finale(phi: int):
                """e3 = (p1^3 - 3 p1 p2 + 2 p3)/6 summed over e, for one
                PSUM fill. p1 arrives parity-presummed (s1 rhs); p2/p3
                carry both f-parity halves."""
                p1t, p2t, p3t = psums[phi]
                v2 = p2t[:].rearrange("p (j t e) -> p j t e", j=J2, t=2)
                # p1^2 first: p1's matmuls finish 8 MMs before p3's, so
                # ACT can square it while p2/p3 still stream.
                t1 = tail_pool.tile([128, J2 * E], f32)
                nc.scalar.square(t1[:], p1t[:])  # p1^2 (PSUM src)
                # u2 = p1^2 - 3*(p2_par0 + p2_par1), folded as two
                # one-PSUM-operand stt ops (no staging copy on the chain).
                w2 = tail_pool.tile([128, J2 * E], f32)
                nc.vector.scalar_tensor_tensor(
                    w2[:], v2[:, :, 0, :], -3.0, t1[:],
                    op0=mybir.AluOpType.mult, op1=mybir.AluOpType.add,
                )
                u2 = tail_pool.tile([128, J2 * E], f32)
                nc.vector.scalar_tensor_tensor(
                    u2[:], v2[:, :, 1, :], -3.0, w2[:],
                    op0=mybir.AluOpType.mult, op1=mybir.AluOpType.add,
                )
                u3 = tail_pool.tile([128, J2 * E], f32)
                nc.vector.tensor_mul(u3[:], u2[:], p1t[:])  # p1^3 - 3 p1 p2
                redu = tail_pool.tile([128, J2], f32)
                nc.vector.reduce_sum(
                    redu[:],
                    u3[:].rearrange("p (j e) -> p j e", j=J2),
                    axis=mybir.AxisListType.X,
                )
                # p3's branch last: it depends on the final matmuls, and
                # the ready u-chain drains off the DVE queue meanwhile.
                # sum_e(u3 + 2*p3) = sum_e(u3) + 2*sum_e(p3).
                # The final /6 happens on the host (bit-exact fp32 mul).
                v3 = p3t[:].rearrange("p (j t e) -> p j t e", j=J2, t=2)
                r3p = tail_pool.tile([128, J2, 2], f32)
                nc.vector.reduce_sum(r3p[:], v3, axis=mybir.AxisListType.X)
                red3 = tail_pool.tile([128, J2], f32)
                nc.vector.tensor_add(red3[:], r3p[:, :, 0], r3p[:, :, 1])
                nc.vector.scalar_tensor_tensor(  # 2*sum(p3) + sum(u3)
                    outt[:, J2 * phi : J2 * (phi + 1)], red3[:], 2.0, redu[:],
                    op0=mybir.AluOpType.mult, op1=mybir.AluOpType.add,
                )
                # one merged store at the end (fewer DMA calls/sems)
                if phi == 1:
                    nc.sync.dma_start(out=out_dram[:], in_=outt[:])

            for s in range(N_SUPER):
                xb = x_pool.tile([128, SFD], bf16)
                # HWDGE loads of the pre-permuted stream: contiguous 4 KB
                # per partition per superblock. First superblocks are
                # split to shorten the pipeline ramp.
                nsplit = SUPER if s == 0 else 1
                csz = SFD // nsplit
                for c in range(nsplit):
                    lo = s * SFD + c * csz
                    # the very first chunk goes out on the scalar HWDGE
                    # ring, whose preamble finishes ~1us before sync's
                    eng = nc.scalar if s == 0 and c == 0 else nc.sync
                    eng.dma_start(
                        out=xb[:, c * csz : (c + 1) * csz],
                        in_=x_dram[:, lo : lo + csz],
                    )
                    if s == 0 and c == 0:
                        # g rides on the scalar ring behind the first x
                        # chunk so the sync ring's x loads start sooner
                        nc.scalar.dma_start(out=g_sb[:], in_=g_dram[:])
                x2b = x2_pool.tile([128, SFD], bf16)
                x3b = x3_pool.tile([128, SFD], bf16)
                s1b = s1_pool.tile([128, SFD // 2], bf16)
                for c in range(nsplit):
                    cs = slice(c * csz, (c + 1) * csz)
                    # parity pre-sum of x first: it only needs xb, so p1's
                    # matmuls can start ~1.4us before p2's (ACT square)
                    v = xb[:, cs].rearrange("p (m t e) -> p m t e", t=2, e=E)
                    s1v = s1b[:, c * csz // 2 : (c + 1) * csz // 2].rearrange(
                        "p (m e) -> p m e", e=E
                    )
                    nc.vector.tensor_add(s1v, v[:, :, 0, :], v[:, :, 1, :])
                    # x^2 whole on ACT: splitting it ACT/DVE to cut ACT's
                    # ~93% utilization measures worse (+4us idle) — the
                    # x^3 op then straddles two producers and stalls on
                    # whichever half lands late.
                    nc.scalar.square(x2b[:, cs], xb[:, cs])
                    nc.vector.tensor_mul(x3b[:, cs], x2b[:, cs], xb[:, cs])
                # Matmul emission order matters: TensorE drains its queue
                # in FIFO order, so an MM waiting on late data blocks
                # ready MMs behind it. For split superblocks (ramp), go
                # chunk-major — each chunk's p1/p2 right as it lands,
                # p3s at the end once the x^3 chain has drained. For
                # steady-state superblocks (data arrives far ahead),
                # stat-major order matches operand readiness.
                phi = s // FILL_SUPERS

                def mm(stat, src, k):
                    fd = FD // 2 if stat == 0 else FD
                    taup = (s % FILL_SUPERS) * SUPER + k
                    m16, cg = taup % 16, taup // 16
                    nc.tensor.matmul(
                        psums[phi][stat][64 * cg : 64 * cg + 64, :],
                        g_sb[:, 60 - 4 * m16 : 124 - 4 * m16],
                        src[:, k * fd : (k + 1) * fd],
                        start=m16 == 0,
                        stop=m16 == 15,
                        skip_group_check=True,
                    )

                if nsplit > 1:
                    tpc = SUPER // nsplit
                    for c in range(nsplit):
                        for k in range(c * tpc, (c + 1) * tpc):
                            mm(0, s1b, k)
                            mm(1, x2b, k)
                    for k in range(SUPER):
                        mm(2, x3b, k)
                else:
                    for stat, src in ((0, s1b), (1, x2b), (2, x3b)):
                        for k in range(SUPER):
                            mm(stat, src, k)
                if s % FILL_SUPERS == FILL_SUPERS - 1:
                    finale(s // FILL_SUPERS)

    nc.compile()
    return nc


def _get_nc():
    if "nc" not in _cache:
        _cache["nc"] = _build()
    return _cache["nc"]


def _permute_shard(shard: np.ndarray) -> np.ndarray:
    """(1024, 64, 64) bf16 -> (128, 32768) in the on-chip stream order:
    out[b_q*32+fp, tau*512 + j2*128 + t*64 + e] = x[16*tau+4*j2+b_q,
    2*fp+t, e]."""
    xp = shard.reshape(TILES, 4, 4, 32, 2, E)  # tau, j2, b_q, fp, t, e
    xp = xp.transpose(2, 3, 0, 1, 4, 5)        # b_q, fp, tau, j2, t, e
    return np.ascontiguousarray(xp).reshape(128, TILES * FD)


def _unpermute(r: np.ndarray) -> np.ndarray:
    # r[4*tau' + b_q, J2*phi + j2] is the value for
    # b = 512*phi + 16*tau' + 4*j2 + b_q
    return np.transpose(r.reshape(32, 4, 2, J2), (2, 0, 3, 1)).reshape(-1)


def _run(x: np.ndarray, **kwargs):
    import ml_dtypes

    from concourse.bass_utils import run_bass_kernel_spmd

    nc = _get_nc()
    g = _make_g()
    shards = x.astype(ml_dtypes.bfloat16).reshape(N_CORES, B_PER_CORE, F, E)
    in_maps = [
        {"x": _permute_shard(shards[c]), "g": g} for c in range(N_CORES)
    ]
    res = run_bass_kernel_spmd(nc, in_maps, core_ids=list(range(N_CORES)), **kwargs)
    out = np.concatenate(
        [_unpermute(np.asarray(res.results[c]["out"])) for c in range(N_CORES)]
    ).astype(np.float32)
    out *= np.float32(1.0 / 6.0)  # the /6 of Newton's formula, host-side
    return out, res


def kernel(**inputs) -> np.ndarray:
    x = np.ascontiguousarray(np.asarray(inputs["x"], dtype=np.float32))
    assert x.shape == (B, F, E), x.shape
    out, _ = _run(x)
    return out
